# revision 1
# baseline (speedup 1.0000x reference)
"""CrossLayerTranscoder with global batch-wise top-k masking on 8 TRN2 cores.

Reference computation:
    pre = relu(x @ W_enc + b_enc)            [4096, 16384]
    keep the global top-(top_k * 4096) entries, zero the rest.

Device algorithm (single pass, dict-sharded over 8 cores), a 4-engine
pipeline per [128 cols x 512 rows] tile:
  * PE: GEMM in single-term bf16 (1 cycle/row; differential noise on
    pre_acts ~1.6e-3 rms - everything accuracy-critical is repaired on
    the host).  Transposed orientation: partition = dict col, free = row.
  * ACT: a = 4096*z + (M + 4096*b) with M = 1.5*2^23.  The f32 store
    rounds to M + q exactly (ulp(M)=1), i.e. q = round(4096*(z+b)) -
    a Round op built from the classic big-constant trick.
  * Pool: packed = (a - M) + iota/512 = q + i/512, exact in f32
    (|q| < 2^15, i < 512 -> 24 mantissa bits).  Value-major,
    index-minor packing.
  * DVE: single MAX8 per tile extracts the top-8 packed values per
    (dict col, 512-row block) - candidate value AND row index in one
    scan.  No MATCH_VALUE_LOAD / FIND_INDEX8 second pass.
  * Global merge on host:
      - decode q = floor(packed), i = (packed-q)*512; v_hat = q/4096.
      - tau_hat = k-th largest candidate.
      - 'saturated' chunks (8th candidate still >= tau_hat - DELTA) may
        hide more top-k members beyond the extracted 8: recompute those
        chunks' dot products exactly on host (~hundreds).
      - candidates within +-DELTA of tau_hat get exact recomputation
        too; exact values decide the top-k boundary, so the selected
        SET matches the reference's (a swapped element costs ~2*tau^2
        in norm^2, which is what the metric is sensitive to).
      - confident candidates (> tau_hat + DELTA) keep the quantized
        device value; its ~1.6e-3 noise is far inside the error budget.
"""

import numpy as np

P = 128
N_TOTAL = 4096
K_DIM = 768
DICT = 16384
N_CORES = 8
DICT_SH = DICT // N_CORES     # 2048
KCH = K_DIM // P              # 6
R_BLK = 512
R_BLOCKS = N_TOTAL // R_BLK   # 8
D_TILES = DICT_SH // P        # 16
CW = 8                        # top-8 per (col, 512-row block)
DGRP = 4                      # d-tiles packed per STT instruction
DELTA = 2.5e-2                # band half-width around tau_hat (~15 sigma)
MROUND = 1.5 * 2.0**23        # fp32 round-to-int magic constant
QSCALE = 4096.0               # value quantization: q = round(4096*(z+b))

_cache = {}


def _build_sparse():
    import concourse.mybir as mybir
    import concourse.tile as tile
    from concourse import bacc

    f32 = mybir.dt.float32
    bf16 = mybir.dt.bfloat16

    nc = bacc.Bacc("TRN2", target_bir_lowering=False, debug=False,
                   num_devices=N_CORES)
    # host pre-arranged layouts: xT[p, c, n] = x.T[c*128+p, n],
    # wh[p, c, n] = W[c*128+p, n]  (partition-major for 1-descriptor DMAs)
    xT = nc.dram_tensor("xT", [P, KCH * N_TOTAL], bf16, kind="ExternalInput")
    wh = nc.dram_tensor("wh", [P, KCH * DICT_SH], bf16, kind="ExternalInput")
    b = nc.dram_tensor("b", [P, D_TILES], f32, kind="ExternalInput")
    cio = nc.dram_tensor("cio", [P, R_BLK], f32, kind="ExternalInput")
    cval = nc.dram_tensor("cval", [R_BLOCKS * P, D_TILES * CW], f32,
                          kind="ExternalOutput")

    with tile.TileContext(nc) as tc:
        with (
            tc.tile_pool(name="resident", bufs=1) as rpool,
            tc.tile_pool(name="xstream", bufs=3) as xpool,
            tc.tile_pool(name="act", bufs=8) as apool,
            tc.tile_pool(name="pack", bufs=8) as ppool,
            tc.tile_pool(name="cand", bufs=2) as cpool,
            tc.tile_pool(name="psum", bufs=8, space="PSUM") as psum_pool,
        ):
            wh_sb = rpool.tile([P, KCH, DICT_SH], bf16)
            b_sb = rpool.tile([P, D_TILES], f32)
            io_sb = rpool.tile([P, R_BLK], f32)

            # x host layout [p][rb][c][rr]: each r-block is one contiguous
            # 6 KB run per partition -> single flat high-bandwidth DMA
            xT_r = xT.ap().rearrange("p (rb c rr) -> p rb c rr",
                                     c=KCH, rr=R_BLK)
            wh_r = wh.ap().rearrange("p (c n) -> p c n", c=KCH)
            cval_r = cval.ap().rearrange("(rb p) w -> p rb w", p=P)

            # priority-ordered head: the first LDW needs W(k=0, cols 0:128)
            # and the first MM additionally x(r0); later W arrives in a
            # ramp that beats the PE's 128-cols-per-1.28us consumption pace
            x0 = xpool.tile([P, KCH, R_BLK], bf16, tag="xh")
            nc.sync.dma_start(wh_sb[:, 0, 0:128], wh_r[:, 0, 0:128])
            nc.sync.dma_start(x0[:], xT_r[:, 0])
            nc.sync.dma_start(wh_sb[:, 1:, 0:128], wh_r[:, 1:, 0:128])
            nc.sync.dma_start(b_sb[:], b.ap())
            nc.sync.dma_start(io_sb[:], cio.ap())
            edges = [128, 256, 512, 1024, 2048]
            for q0, q1 in zip(edges[:-1], edges[1:]):
                nc.sync.dma_start(wh_sb[:, :, q0:q1], wh_r[:, :, q0:q1])

            # x prefetch distance 1: issue x(r+1) at the TOP of block r's
            # body, ahead of cval(r) in the in-order sync queue - otherwise
            # cval(r)'s semaphore wait (end of block r's DVE) delays the
            # x(r+1) transfer into block r+1's compute window
            x_next = {0: x0}
            for r in range(R_BLOCKS):
                xh_t = x_next.pop(r)
                if r + 1 < R_BLOCKS:
                    xn = xpool.tile([P, KCH, R_BLK], bf16, tag="xh")
                    nc.sync.dma_start(xn[:], xT_r[:, r + 1])
                    x_next[r + 1] = xn
                cvb = cpool.tile([P, D_TILES, CW], f32, tag="cv")
                for d in range(D_TILES):
                    ps = psum_pool.tile([P, R_BLK], mybir.dt.float32)
                    dsl = slice(d * P, (d + 1) * P)
                    for k in range(KCH):
                        nc.tensor.matmul(
                            ps[:], wh_sb[:, k, dsl], xh_t[:, k, :],
                            start=(k == 0), stop=(k == KCH - 1))
                    a_sb = apool.tile([P, R_BLK], f32, tag="a")
                    nc.scalar.activation(
                        a_sb[:], ps[:],
                        mybir.ActivationFunctionType.Identity,
                        bias=b_sb[:, d:d + 1], scale=QSCALE)
                    pk = ppool.tile([P, R_BLK], f32, tag="pk")
                    nc.vector.scalar_tensor_tensor(
                        pk[:], a_sb[:], MROUND, io_sb[:],
                        op0=mybir.AluOpType.subtract,
                        op1=mybir.AluOpType.add)
                    nc.vector.max(cvb[:, d], pk[:])
                nc.sync.dma_start(cval_r[:, r], cvb[:])
    nc.compile()
    return nc


def _get_kernel():
    if "k" not in _cache:
        _cache["k"] = _build_sparse()
    return _cache["k"]


def kernel(x, W_enc, b_enc, top_k):
    import ml_dtypes
    from concourse.bass_utils import run_bass_kernel_spmd

    x = np.ascontiguousarray(np.asarray(x), np.float32)
    W_enc = np.ascontiguousarray(np.asarray(W_enc), np.float32)
    b_enc = np.ascontiguousarray(np.asarray(b_enc), np.float32).ravel()
    top_k = int(np.asarray(top_k))
    k_tot = top_k * x.shape[0]
    out = np.zeros((N_TOTAL, DICT), np.float32)
    if k_tot <= 0:
        return out

    nc = _get_kernel()

    # partition-major host layouts: [p, c, n]
    xTp = np.ascontiguousarray(
        x.T.astype(ml_dtypes.bfloat16).reshape(KCH, P, R_BLOCKS, R_BLK)
        .transpose(1, 2, 0, 3).reshape(P, KCH * N_TOTAL))
    W16 = W_enc.astype(ml_dtypes.bfloat16)
    iot = np.tile((np.arange(R_BLK, dtype=np.float32) / 512.0)[None, :],
                  (P, 1))
    ins = []
    for c in range(N_CORES):
        sl = slice(c * DICT_SH, (c + 1) * DICT_SH)
        whp = np.ascontiguousarray(
            W16[:, sl].reshape(KCH, P, DICT_SH)
            .transpose(1, 0, 2).reshape(P, KCH * DICT_SH))
        bsh = (MROUND + QSCALE *
               np.ascontiguousarray(b_enc[sl]).reshape(D_TILES, P).T
               ).astype(np.float32).copy()
        ins.append({"xT": xTp, "wh": whp, "b": bsh, "cio": iot})

    try:
        res = run_bass_kernel_spmd(nc, ins, core_ids=list(range(N_CORES)))
    except Exception:
        # transient device errors (e.g. NRT_EXEC_UNIT_UNRECOVERABLE) recover
        # on re-execution; one retry
        res = run_bass_kernel_spmd(nc, ins, core_ids=list(range(N_CORES)))

    # ---- global merge (host) ----
    # flat layout: [core, rb, p, d, slot];  col = c*2048 + d*128 + p,
    # row = rb*512 + i;  packed = q + i/512, q = round(4096*(z+b))
    vals = np.stack([res.results[c]["cval"] for c in range(N_CORES)])
    packed = vals.ravel().astype(np.float64)
    q = np.floor(packed)
    ii = np.rint((packed - q) * 512.0).astype(np.int64)
    vb = (q / QSCALE).astype(np.float64)

    n_flat = packed.size
    f = np.arange(n_flat, dtype=np.int64)
    c_, rem = np.divmod(f, R_BLOCKS * P * D_TILES * CW)
    rb, rem = np.divmod(rem, P * D_TILES * CW)
    p, rem = np.divmod(rem, D_TILES * CW)
    d, slot = np.divmod(rem, CW)
    col = (c_ * DICT_SH + d * P + p).astype(np.int64)
    row = rb * R_BLK + ii

    k_eff = min(k_tot, n_flat)
    tau_hat = float(np.partition(vb, -k_eff)[-k_eff])

    if tau_hat <= DELTA:
        # degenerate regime (k >= positive count): values near zero,
        # approximate selection is fine
        keep = vb > 0
        order = np.argsort(-vb[keep])[:k_tot]
        out[row[keep][order], col[keep][order]] = vb[keep][order]
        return out

    # chunk = (core, rb, p, d) <-> flat // CW; slot 7 is the chunk's 8th
    # (smallest extracted) value: if it is still near/above the threshold
    # the chunk may hide more top-k members beyond the extracted 8.
    v8 = vb[slot == 7]
    sat_chunk = np.flatnonzero(v8 >= tau_hat - DELTA)   # chunk ids
    chunk_id = f // CW
    in_sat = np.isin(chunk_id, sat_chunk)

    conf = (vb > tau_hat + DELTA) & ~in_sat
    band = (vb >= tau_hat - DELTA) & (vb <= tau_hat + DELTA) & ~in_sat

    # exact recompute pool: all rows of saturated chunks + band candidates
    er_list = [row[band]]
    ec_list = [col[band]]
    if sat_chunk.size:
        sc_, srem = np.divmod(sat_chunk, R_BLOCKS * P * D_TILES)
        srb, srem = np.divmod(srem, P * D_TILES)
        sp, sd = np.divmod(srem, D_TILES)
        scol = sc_ * DICT_SH + sd * P + sp
        er_list.append(
            (srb[:, None] * R_BLK + np.arange(R_BLK)[None, :]).ravel())
        ec_list.append(np.repeat(scol, R_BLK))
    er = np.concatenate(er_list)
    ec = np.concatenate(ec_list)
    # dedupe exact positions
    epos = er * DICT + ec
    epos, uq = np.unique(epos, return_index=True)
    er, ec = er[uq], ec[uq]

    ev = np.empty(er.size, np.float64)
    CH = 65536
    for i in range(0, er.size, CH):
        s = slice(i, i + CH)
        ev[s] = np.einsum(
            "ij,ij->i",
            x[er[s]].astype(np.float64),
            W_enc[:, ec[s]].T.astype(np.float64),
            optimize=True) + b_enc[ec[s]]

    # confident candidates are all truly in the top-k (their true value is
    # > tau_hat + DELTA - noise > tau); duplicated positions carry
    # identical values, so plain assignment is safe
    out[row[conf], col[conf]] = vb[conf].astype(np.float32)
    n_conf = np.unique(row[conf] * DICT + col[conf]).size

    need = k_tot - n_conf
    if need > 0:
        # exact values decide the boundary; ties -> lowest flat index,
        # matching jax.lax.top_k
        order = np.lexsort((epos, -ev.astype(np.float64)))
        kept = order[:need]
        out[er[kept], ec[kept]] = np.maximum(ev[kept], 0)
    return out



# revision 2
# speedup vs baseline: 1.6388x; 1.6388x over previous
"""CrossLayerTranscoder with global batch-wise top-k masking on 8 TRN2 cores.

Reference computation:
    pre = relu(x @ W_enc + b_enc)            [4096, 16384]
    keep the global top-(top_k * 4096) entries, zero the rest.

Device algorithm (dict-sharded over 8 cores), per [128 cols x 512 rows] tile:
  * PE: GEMM in fp8(e4m3) with perf_mode=DoubleRow - each matmul absorbs
    K=256 contraction rows at ~2 rows/cycle, halving PE time vs bf16.
    W is pre-scaled by 32 (power of two) to center its values in the fp8
    normal range; the ACT scale divides it back out.  Differential noise
    on z is ~0.05 rms - all accuracy-critical values are recomputed
    exactly on the host.
  * ACT: a = 8*psum + (M + 256*b) stored f32; with M = 1.5*2^23 the store
    rounds to M + q exactly (ulp(M)=1), q = round(256*(z+b)).
  * pk = (a - M) + i/512: value-major, index-minor packing, exact in f32.
    Engine-split to balance load: even d-tiles run it as one DVE
    scalar_tensor_tensor; odd d-tiles run ACT (a - M) then GPSIMD
    tensor_tensor (+ i/512), freeing DVE for the MAX8s.
  * DVE: MAX8 per tile extracts the top-8 packed values per
    (dict col, 512-row block) - candidate value AND row index in one scan.
  * Host merge:
      - decode q = floor(packed), i = (packed-q)*512; v_hat = q/256.
      - tau_hat = k-th largest candidate.
      - candidate pool: all candidates with v_hat >= tau_hat - DELTA, plus
        every entry of 'saturated' chunks (8th extracted value still >=
        tau_hat - DELTA, so more members may hide beyond the top 8).
        Saturated chunks are recomputed with per-row-block BLAS GEMMs;
        the rest of the pool with a chunked einsum.  All pool values are
        EXACT (f64) - both the selected set and the stored values come
        from exact arithmetic, so fp8 device noise never reaches the
        output.
"""

import numpy as np

P = 128
N_TOTAL = 4096
K_DIM = 768
DICT = 16384
N_CORES = 8
DICT_SH = DICT // N_CORES     # 2048
R_BLK = 512
R_BLOCKS = N_TOTAL // R_BLK   # 8
D_TILES = DICT_SH // P        # 16
CW = 8                        # top-8 per (col, 512-row block)
KP = K_DIM // 256             # 3 DoubleRow k-pairs
DELTA = 0.26                  # band half-width (~5 sigma of fp8 noise)
MROUND = 1.5 * 2.0**23        # fp32 round-to-int magic constant
QSCALE = 256.0                # value quantization: q = round(256*(z+b))
W_SCALE = 32.0                # fp8 pre-scale on W (power of 2)

_cache = {}


def _build_sparse():
    import concourse.mybir as mybir
    import concourse.tile as tile
    from concourse import bacc

    f32 = mybir.dt.float32
    fp8 = mybir.dt.float8e4

    nc = bacc.Bacc("TRN2", target_bir_lowering=False, debug=False,
                   num_devices=N_CORES)
    # host layouts (partition-major):
    #   x8[p, rb, c2, ko, rr] = fp8(x[c2*256 + ko*128 + p, rb*512 + rr])
    #   w8[p, c2, ko, m]      = fp8(32 * W[c2*256 + ko*128 + p, m])
    x8 = nc.dram_tensor("x8", [P, R_BLOCKS * KP * 2 * R_BLK], fp8,
                        kind="ExternalInput")
    w8 = nc.dram_tensor("w8", [P, KP * 2 * DICT_SH], fp8,
                        kind="ExternalInput")
    b = nc.dram_tensor("b", [P, D_TILES], f32, kind="ExternalInput")
    mneg = nc.dram_tensor("mneg", [P, 1], f32, kind="ExternalInput")
    cio = nc.dram_tensor("cio", [P, R_BLK], f32, kind="ExternalInput")
    cval = nc.dram_tensor("cval", [R_BLOCKS * P, D_TILES * CW], f32,
                          kind="ExternalOutput")

    with tile.TileContext(nc) as tc:
        with (
            tc.tile_pool(name="resident", bufs=1) as rpool,
            tc.tile_pool(name="xstream", bufs=3) as xpool,
            tc.tile_pool(name="act", bufs=8) as apool,
            tc.tile_pool(name="pack", bufs=8) as ppool,
            tc.tile_pool(name="cand", bufs=2) as cpool,
            tc.tile_pool(name="psum", bufs=8, space="PSUM") as psum_pool,
        ):
            w8_sb = rpool.tile([P, KP, 2, DICT_SH], fp8)
            b_sb = rpool.tile([P, D_TILES], f32)
            mn_sb = rpool.tile([P, 1], f32)
            io_sb = rpool.tile([P, R_BLK], f32)

            x8_r = x8.ap().rearrange("p (rb c k rr) -> p rb c k rr",
                                     c=KP, k=2, rr=R_BLK)
            w8_r = w8.ap().rearrange("p (c k m) -> p c k m", c=KP, k=2)
            cval_r = cval.ap().rearrange("(rb p) w -> p rb w", p=P)

            # priority-ordered head: first MMs need w8 cols 0:128 (all
            # kpairs) and x8(r0); later W cols arrive in a ramp
            x0 = xpool.tile([P, KP, 2, R_BLK], fp8, tag="xh")
            nc.sync.dma_start(w8_sb[:, :, :, 0:128], w8_r[:, :, :, 0:128])
            nc.sync.dma_start(x0[:], x8_r[:, 0])
            nc.sync.dma_start(b_sb[:], b.ap())
            nc.sync.dma_start(mn_sb[:], mneg.ap())
            nc.sync.dma_start(io_sb[:], cio.ap())
            edges = [128, 256, 512, 1024, 2048]
            for q0, q1 in zip(edges[:-1], edges[1:]):
                nc.sync.dma_start(w8_sb[:, :, :, q0:q1], w8_r[:, :, :, q0:q1])

            x_next = {0: x0}
            for r in range(R_BLOCKS):
                xh_t = x_next.pop(r)
                if r + 1 < R_BLOCKS:
                    xn = xpool.tile([P, KP, 2, R_BLK], fp8, tag="xh")
                    nc.sync.dma_start(xn[:], x8_r[:, r + 1])
                    x_next[r + 1] = xn
                cvb = cpool.tile([P, D_TILES, CW], f32, tag="cv")
                for d in range(D_TILES):
                    ps = psum_pool.tile([P, R_BLK], mybir.dt.float32)
                    dsl = slice(d * P, (d + 1) * P)
                    for c2 in range(KP):
                        nc.tensor.matmul(
                            ps[:], w8_sb[:, c2, :, dsl], xh_t[:, c2],
                            start=(c2 == 0), stop=(c2 == KP - 1),
                            perf_mode=mybir.MatmulPerfMode.DoubleRow)
                    a_sb = apool.tile([P, R_BLK], f32, tag="a")
                    nc.scalar.activation(
                        a_sb[:], ps[:],
                        mybir.ActivationFunctionType.Identity,
                        bias=b_sb[:, d:d + 1], scale=QSCALE / W_SCALE)
                    pk = ppool.tile([P, R_BLK], f32, tag="pk")
                    if d % 2 == 0:
                        # class D: fused on DVE
                        nc.vector.scalar_tensor_tensor(
                            pk[:], a_sb[:], MROUND, io_sb[:],
                            op0=mybir.AluOpType.subtract,
                            op1=mybir.AluOpType.add)
                    else:
                        # class G: ACT removes M, GPSIMD adds the iota
                        a2 = apool.tile([P, R_BLK], f32, tag="a2")
                        nc.scalar.activation(
                            a2[:], a_sb[:],
                            mybir.ActivationFunctionType.Identity,
                            bias=mn_sb[:, 0:1], scale=1.0)
                        nc.gpsimd.tensor_tensor(
                            pk[:], a2[:], io_sb[:], op=mybir.AluOpType.add)
                    nc.vector.max(cvb[:, d], pk[:])
                nc.sync.dma_start(cval_r[:, r], cvb[:])
    nc.compile()
    return nc


def _get_kernel():
    if "k" not in _cache:
        _cache["k"] = _build_sparse()
    return _cache["k"]


def prepare_inputs(x, W_enc, b_enc):
    """Build the per-core device input dicts from full f32 inputs."""
    import ml_dtypes

    fp8 = ml_dtypes.float8_e4m3

    # x8[p, rb, c2, ko, rr]: x is [N, K]; contraction index
    # k = c2*256 + ko*128 + p
    xT = np.ascontiguousarray(x.T)                       # [768, 4096]
    x8h = np.ascontiguousarray(
        xT.reshape(KP, 2, P, R_BLOCKS, R_BLK)
        .transpose(2, 3, 0, 1, 4)
        .reshape(P, R_BLOCKS * KP * 2 * R_BLK)).astype(fp8)
    iot = np.tile((np.arange(R_BLK, dtype=np.float32) / 512.0)[None, :],
                  (P, 1))
    mn = np.full((P, 1), -MROUND, np.float32)
    ins = []
    for c in range(N_CORES):
        sl = slice(c * DICT_SH, (c + 1) * DICT_SH)
        w8h = np.ascontiguousarray(
            (W_SCALE * W_enc[:, sl]).reshape(KP, 2, P, DICT_SH)
            .transpose(2, 0, 1, 3)
            .reshape(P, KP * 2 * DICT_SH)).astype(fp8)
        bsh = (MROUND + QSCALE *
               np.ascontiguousarray(b_enc[sl]).reshape(D_TILES, P).T
               ).astype(np.float32).copy()
        ins.append({"x8": x8h, "w8": w8h, "b": bsh, "mneg": mn, "cio": iot})
    return ins


def _host_fallback(x, W_enc, b_enc, k_tot):
    """Exact dense path; only for degenerate/unexpected regimes."""
    z = x.astype(np.float32) @ W_enc.astype(np.float32) + b_enc[None, :]
    z = np.maximum(z, 0.0)
    flat = z.reshape(-1)
    idx = np.argpartition(-flat, k_tot - 1)[:k_tot]
    order = np.lexsort((idx, -flat[idx]))
    idx = idx[order]
    out = np.zeros_like(flat)
    out[idx] = flat[idx]
    return out.reshape(z.shape)


def kernel(x, W_enc, b_enc, top_k):
    from concourse.bass_utils import run_bass_kernel_spmd

    x = np.ascontiguousarray(np.asarray(x), np.float32)
    W_enc = np.ascontiguousarray(np.asarray(W_enc), np.float32)
    b_enc = np.ascontiguousarray(np.asarray(b_enc), np.float32).ravel()
    top_k = int(np.asarray(top_k))
    k_tot = top_k * x.shape[0]
    out = np.zeros((N_TOTAL, DICT), np.float32)
    if k_tot <= 0:
        return out

    nc = _get_kernel()
    ins = prepare_inputs(x, W_enc, b_enc)
    try:
        res = run_bass_kernel_spmd(nc, ins, core_ids=list(range(N_CORES)))
    except Exception:
        # transient device errors recover on re-execution; one retry
        res = run_bass_kernel_spmd(nc, ins, core_ids=list(range(N_CORES)))

    # ---- global merge (host) ----
    # flat layout: [core, rb, p, d, slot];  col = c*2048 + d*128 + p,
    # row = rb*512 + i;  packed = q + i/512, q = round(256*(z+b))
    vals = np.stack([res.results[c]["cval"] for c in range(N_CORES)])
    packed = vals.ravel().astype(np.float64)
    q = np.floor(packed)
    ii = np.rint((packed - q) * 512.0).astype(np.int64)
    vb = q / QSCALE

    n_flat = packed.size
    f = np.arange(n_flat, dtype=np.int64)
    c_, rem = np.divmod(f, R_BLOCKS * P * D_TILES * CW)
    rb, rem = np.divmod(rem, P * D_TILES * CW)
    p, rem = np.divmod(rem, D_TILES * CW)
    d, slot = np.divmod(rem, CW)
    col = (c_ * DICT_SH + d * P + p).astype(np.int64)
    row = rb * R_BLK + ii

    k_eff = min(k_tot, n_flat)
    tau_hat = float(np.partition(vb, -k_eff)[-k_eff])
    if tau_hat <= 2.0 * DELTA:
        # degenerate regime (huge k / tiny tau): device extraction cannot
        # cover the selection; fall back to the exact dense path.
        return _host_fallback(x, W_enc, b_enc, k_tot)

    thr = tau_hat - DELTA
    x64 = x.astype(np.float64)
    W64_cols = W_enc  # column gather done lazily below
    b64 = b_enc.astype(np.float64)

    # pool of exact values keyed by flat position
    pool_pos = []
    pool_val = []

    # 1. saturated chunks: 8th extracted value still above threshold ->
    #    recompute the whole (col, rb) chunk with per-rb BLAS GEMMs.
    v8 = vb[slot == 7]                       # per chunk (c, rb, p, d)
    sat_chunk = np.flatnonzero(v8 >= thr)
    in_sat = np.zeros(0, np.int64)
    if sat_chunk.size:
        sc_, srem = np.divmod(sat_chunk, R_BLOCKS * P * D_TILES)
        srb, srem = np.divmod(srem, P * D_TILES)
        sp, sd = np.divmod(srem, D_TILES)
        scol = sc_ * DICT_SH + sd * P + sp
        for rbi in range(R_BLOCKS):
            m = srb == rbi
            if not m.any():
                continue
            cols = np.unique(scol[m])
            xa = x64[rbi * R_BLK:(rbi + 1) * R_BLK]         # [512, 768]
            zc = xa @ W64_cols[:, cols].astype(np.float64)  # [512, ncols]
            zc += b64[cols][None, :]
            rr, cc = np.nonzero(zc >= thr)
            if rr.size:
                pool_pos.append((rbi * R_BLK + rr) * DICT + cols[cc])
                pool_val.append(zc[rr, cc])
        chunk_id = f // CW
        in_sat_mask = np.isin(chunk_id, sat_chunk)
    else:
        in_sat_mask = np.zeros(n_flat, bool)

    # 2. remaining candidates above threshold: exact einsum recompute
    cand = (vb >= thr) & ~in_sat_mask
    er, ec = row[cand], col[cand]
    epos = er * DICT + ec
    epos, uq = np.unique(epos, return_index=True)
    er, ec = er[uq], ec[uq]
    ev = np.empty(er.size, np.float64)
    CH = 65536
    for i in range(0, er.size, CH):
        s = slice(i, i + CH)
        ev[s] = np.einsum(
            "ij,ij->i",
            x64[er[s]],
            W_enc[:, ec[s]].T.astype(np.float64)) + b64[ec[s]]
    pool_pos.append(epos)
    pool_val.append(ev)

    ppos = np.concatenate(pool_pos)
    pval = np.concatenate(pool_val)
    # dedupe (sat slab and einsum could overlap only across different
    # chunks, but be safe)
    ppos, uq = np.unique(ppos, return_index=True)
    pval = pval[uq]

    if pval.size < k_tot:
        return _host_fallback(x, W_enc, b_enc, k_tot)

    # exact selection: value desc, flat index asc (matches jax.lax.top_k)
    order = np.lexsort((ppos, -pval))
    kept = order[:k_tot]
    kr, kc = np.divmod(ppos[kept], DICT)
    out[kr, kc] = np.maximum(pval[kept], 0.0).astype(np.float32)
    return out
